# revision 22
# baseline (speedup 1.0000x reference)
"""Centered locally-connected 1x1 conv on 8 TRN2 NeuronCores.

Math (G=1 squeezed):
    out_s[b,j,h,w] = sum_i (x+b)[b,i,h,w] * w[i,j,h,w]
    m[b,j]         = (1/(H*W)) * sum_{i,h,w} b[b,i,h,w] * w[i,j,h,w]
    out            = out_s - m

Sharding: H split across the 8 cores (6 rows each); every (h,w) location is an
independent [CI]x[CI,CO] contraction, so each core reads only its slice.  The
spatial mean of the b-path needs a cross-core reduction of a [CO,B] partial
sum (16 KB AllReduce).

Precision (v2): weights ship as fp8 e3m4 with ONE global scale W_SC =
max|w|/15.5 folded into the moving operands on the host.  The PE consumes
fp8e3 stationary tiles directly (mixed-dtype matmul with an fp16 moving
operand verified bit-exact on HW), so there is NO int8->fp16 upcast on the
vector engines — in the v1 int8 kernel that upcast (4.7M elems) saturated
DVE+ACT at ~26us each and was the real co-bottleneck with DMA.
  s-path: moving = W_SC*(x+b) in fp16 -> psum is directly the true product.
  b-path: moving = 8*W_SC*b in fp8e3 (mean path tolerates the ~2% fp8 error
  since it averages over 2304 locations); all 48 locations of a chunk
  accumulate into ONE [128,32] psum bank (start/stop flags), killing the v1
  per-group DVE reduces (15.4us).  Final scale 1/(8*H*W) is data-independent.
Measured emulation error vs fp32 reference: 1.43e-2 (tolerance 2e-2).

Per-core traffic: in 4.72MB w8 + 2.36MB s16 + 1.18MB b8 = 8.26MB, out 2.36MB
fp16 -> ~29.5us of DMA-pool time at the ~360GB/s per-core HBM rate; engines
are far below that (ACT/DVE psum sweeps ~9us, PE ~8us), so the kernel is
DMA-bound end to end.

Schedule (order="bfirst", the default): stream [w,b] for all 6 chunks first
(5.9MB -> b-path complete ~16.5us), accumulating ALL 288 b-matmuls into one
psum bank; drain+scale in one DVE op and kick the 16KB AllReduce (~6.5us
round-trip, measured) immediately — it completes right as the s input stream
(2.36MB) and the weight-resident s-matmuls/psum sweeps finish, so the
collective hides almost entirely.  Then per chunk: broadcast-subtract the
mean (stride-0 AP) and DMA out, pipelined.  The tail is the 2.36MB output
DMA itself.

Measured (interleaved paired-median rep slope, 8-core SPMD): v1 int8+upcast
65.8us -> v2 fp8-direct 38us -> v3 (single b-accumulator, split sweeps,
bfirst) ~31us, vs a ~31us structural floor (lead-in + 8.26MB in + mean-gated
bubble + 2.36MB out).  Probes: input window alone 22.5us; AllReduce
round-trip 3-7us; nocc (no collective) ~33us.
"""

import os
from contextlib import ExitStack

import ml_dtypes
import numpy as np

import concourse.bass as bass
import concourse.mybir as mybir
import concourse.tile as tile
from concourse import bacc
from concourse.bass_utils import run_bass_kernel_spmd

B, CI, H, W, CO = 32, 128, 48, 48, 128
NCORES = 8
HL = H // NCORES          # 6 h-rows per core
LOC = HL * W              # 288 locations per core
CHUNK_L = W               # 48 locations (one h-row) per chunk
NCHUNK = LOC // CHUNK_L   # 6 chunks
KB = 8.0                  # b-path pre-scale, folded into SCALE_M
SCALE_M = 1.0 / (KB * H * W)
F8MAX = 15.5              # e3m4 max

F32 = mybir.dt.float32
F16 = mybir.dt.float16
F8 = mybir.dt.float8e3
E3M4 = ml_dtypes.float8_e3m4

LAST_EXEC_TIME_NS = None
DEFAULT_ORDER = "bfirst"
_NC_CACHE = {}


def _build_nc(
    reps: int = 1,
    mode: str = "full",
    serialize: bool = False,
    order: str = "inter",
    loop_reps: int = 0,
):
    # mode: "in" = input DMAs only; "mm" = +matmuls; "compute" = +ACT/DVE
    #       evacuation; "nocc" = everything but the AllReduce (wrong mean,
    #       perf probe); "full" = the real kernel.
    # order: "inter"  = per-chunk [w,b,s] streams, shared-LDW matmul pairs,
    #                   AllReduce at the very end (simple, AR tail exposed);
    #        "bfirst" = stream all [w,b] chunks first and run the b-path
    #                   immediately, kick the AllReduce at ~2/3 of the DMA
    #                   window, then stream s and run the s-path under the
    #                   collective's latency (weights stay resident; costs a
    #                   second Ldweights per location, PE has the headroom).
    WC = CHUNK_L * 128        # w cols per chunk (fp8)
    SC = CHUNK_L * 32         # s moving cols per chunk (fp16)
    BC = CHUNK_L * 32         # b moving cols per chunk (fp8)

    nc = bacc.Bacc(None)
    w8_d = nc.declare_dram_parameter("w8", [128, NCHUNK * WC], F8, isOutput=False)
    mvs_d = nc.declare_dram_parameter("mvs", [128, NCHUNK * SC], F16, isOutput=False)
    mvb_d = nc.declare_dram_parameter("mvb", [128, NCHUNK * BC], F8, isOutput=False)
    out_d = nc.declare_dram_parameter("out", [128, LOC * 32], F16, isOutput=True)

    with tile.TileContext(nc) as tc, ExitStack() as ctx:
        nbw = 7 if order == "bfirst" else 3
        wp = ctx.enter_context(tc.tile_pool(name="wp", bufs=nbw))
        msp = ctx.enter_context(
            tc.tile_pool(name="msp", bufs=7 if order == "bfirst" else 3)
        )
        mbp = ctx.enter_context(tc.tile_pool(name="mbp", bufs=3))
        spp = ctx.enter_context(tc.tile_pool(name="spp", bufs=2, space="PSUM"))
        bpp = ctx.enter_context(tc.tile_pool(name="bpp", bufs=2, space="PSUM"))
        ocp = ctx.enter_context(tc.tile_pool(name="ocp", bufs=2))
        sp = ctx.enter_context(tc.tile_pool(name="sp", bufs=2))
        dp = ctx.enter_context(tc.tile_pool(name="dp", bufs=2, space="DRAM"))

        def s_matmuls(w_t, ms_t, sp_t):
            for l in range(CHUNK_L):
                nc.tensor.matmul(
                    sp_t[:, l * 32 : (l + 1) * 32],
                    lhsT=w_t[:, l * 128 : (l + 1) * 128],
                    rhs=ms_t[:, l * 32 : (l + 1) * 32],
                    start=True,
                    stop=True,
                    skip_group_check=True,
                )

        def b_matmuls(w_t, mb_t, bp_t, first, last):
            for l in range(CHUNK_L):
                nc.tensor.matmul(
                    bp_t[:, 0:32],
                    lhsT=w_t[:, l * 128 : (l + 1) * 128],
                    rhs=mb_t[:, l * 32 : (l + 1) * 32],
                    start=(first and l == 0),
                    stop=(last and l == CHUNK_L - 1),
                    skip_group_check=True,
                )

        def b_finish(r, bp_t):
            # drain the single b accumulator, scaled by 1/(KB*H*W), in one
            # DVE pass (shortens the serial chain ahead of the AllReduce)
            msc_t = sp.tile([128, 32], F32, name=f"msc{r}", tag="msc")
            nc.vector.tensor_scalar_mul(msc_t[:], bp_t[:, 0:32], SCALE_M)
            if mode == "nocc":
                return msc_t
            # AllReduce across the 8 cores (16 KB)
            cc_in = dp.tile([128, 32], F32, name=f"ci{r}", tag="ci")
            cc_out = dp.tile(
                [128, 32], F32, addr_space="Shared", name=f"co{r}", tag="co"
            )
            nc.sync.dma_start(cc_in[:], msc_t[:])
            nc.gpsimd.collective_compute(
                "AllReduce",
                mybir.AluOpType.add,
                replica_groups=[list(range(NCORES))],
                ins=[cc_in.opt()],
                outs=[cc_out.opt()],
            )
            msum_t = sp.tile([128, 32], F32, name=f"ms{r}", tag="msm")
            nc.sync.dma_start(msum_t[:], cc_out[:])
            return msum_t

        def sub_and_store(c, oc_t, m16_t):
            seg = oc_t[:, c * CHUNK_L * 32 : (c + 1) * CHUNK_L * 32].rearrange(
                "p (r n) -> p r n", n=32
            )
            nc.vector.tensor_sub(
                seg, seg, m16_t[:].unsqueeze(1).to_broadcast((128, CHUNK_L, 32))
            )
            nc.sync.dma_start(
                out_d[:, c * CHUNK_L * 32 : (c + 1) * CHUNK_L * 32],
                oc_t[:, c * CHUNK_L * 32 : (c + 1) * CHUNK_L * 32],
            )

        def body_inter(r):
            oc_t = ocp.tile([128, LOC * 32], F16, name=f"oc{r}", tag="oc")
            bp_t = bpp.tile([128, 512], F32, name=f"bq{r}", tag="bq")
            for c in range(NCHUNK):
                w_t = wp.tile([128, WC], F8, name=f"w{r}_{c}", tag="w")
                nc.sync.dma_start(w_t[:], w8_d[:, c * WC : (c + 1) * WC])
                mb_t = mbp.tile([128, BC], F8, name=f"mb{r}_{c}", tag="mb")
                nc.sync.dma_start(mb_t[:], mvb_d[:, c * BC : (c + 1) * BC])
                ms_t = msp.tile([128, SC], F16, name=f"ms{r}_{c}", tag="ms")
                nc.sync.dma_start(ms_t[:], mvs_d[:, c * SC : (c + 1) * SC])
                if mode == "in":
                    continue
                sp_t = spp.tile([128, CHUNK_L * 32], F32, name=f"sp{r}_{c}", tag="sp")
                for l in range(CHUNK_L):
                    nc.tensor.matmul(
                        sp_t[:, l * 32 : (l + 1) * 32],
                        lhsT=w_t[:, l * 128 : (l + 1) * 128],
                        rhs=ms_t[:, l * 32 : (l + 1) * 32],
                        start=True,
                        stop=True,
                    )
                    nc.tensor.matmul(
                        bp_t[:, 0:32],
                        lhsT=w_t[:, l * 128 : (l + 1) * 128],
                        rhs=mb_t[:, l * 32 : (l + 1) * 32],
                        start=(c == 0 and l == 0),
                        stop=(c == NCHUNK - 1 and l == CHUNK_L - 1),
                        skip_group_check=True,
                    )
                if mode == "mm":
                    continue
                nc.scalar.copy(
                    oc_t[:, c * CHUNK_L * 32 : (c + 1) * CHUNK_L * 32], sp_t[:]
                )
            if mode in ("in", "mm"):
                return
            msum_t = b_finish(r, bp_t)
            if mode == "compute":
                return
            m16_t = sp.tile([128, 32], F16, name=f"m16{r}", tag="m16")
            nc.vector.tensor_copy(out=m16_t[:], in_=msum_t[:])
            for c in range(NCHUNK):
                sub_and_store(c, oc_t, m16_t)

        def body_bfirst(r):
            # Phase 1: stream [w,b] per chunk, run the b-path immediately,
            # kick the AllReduce as soon as the last w/b chunk is consumed.
            # Phase 2: stream s, run the s-path (weights resident) under the
            # collective's latency; subtract+store per chunk as sweeps land.
            oc_t = ocp.tile([128, LOC * 32], F16, name=f"oc{r}", tag="oc")
            bp_t = bpp.tile([128, 512], F32, name=f"bq{r}", tag="bq")
            w_ts = []
            for c in range(NCHUNK):
                w_t = wp.tile([128, WC], F8, name=f"w{r}_{c}", tag="w")
                nc.sync.dma_start(w_t[:], w8_d[:, c * WC : (c + 1) * WC])
                w_ts.append(w_t)
                mb_t = mbp.tile([128, BC], F8, name=f"mb{r}_{c}", tag="mb")
                nc.sync.dma_start(mb_t[:], mvb_d[:, c * BC : (c + 1) * BC])
                if mode == "in":
                    continue
                b_matmuls(w_t, mb_t, bp_t, c == 0, c == NCHUNK - 1)
            ms_ts = []
            for c in range(NCHUNK):
                ms_t = msp.tile([128, SC], F16, name=f"ms{r}_{c}", tag="ms")
                nc.sync.dma_start(ms_t[:], mvs_d[:, c * SC : (c + 1) * SC])
                ms_ts.append(ms_t)
            if mode == "in":
                return
            msum_t = None if mode == "mm" else b_finish(r, bp_t)
            m16_t = None
            if mode not in ("mm", "compute"):
                m16_t = sp.tile([128, 32], F16, name=f"m16{r}", tag="m16")
                nc.vector.tensor_copy(out=m16_t[:], in_=msum_t[:])
            for c in range(NCHUNK):
                sp_t = spp.tile([128, CHUNK_L * 32], F32, name=f"sp{r}_{c}", tag="sp")
                s_matmuls(w_ts[c], ms_ts[c], sp_t)
                if mode == "mm":
                    continue
                # psum sweep: split between ACT and DVE so the trailing
                # chunks' sweeps pipeline instead of queueing on one engine
                oc_seg = oc_t[:, c * CHUNK_L * 32 : (c + 1) * CHUNK_L * 32]
                if c in (1, 3):
                    nc.vector.tensor_copy(out=oc_seg, in_=sp_t[:])
                else:
                    nc.scalar.copy(oc_seg, sp_t[:])
                if mode in ("compute",):
                    continue
                sub_and_store(c, oc_t, m16_t)

        def body_cc(r):
            # collective-only probe: measures the AllReduce round-trip cost
            msc_t = sp.tile([128, 32], F32, name=f"msc{r}", tag="msc")
            nc.vector.memset(msc_t[:], 0.0)
            cc_in = dp.tile([128, 32], F32, name=f"ci{r}", tag="ci")
            cc_out = dp.tile(
                [128, 32], F32, addr_space="Shared", name=f"co{r}", tag="co"
            )
            nc.sync.dma_start(cc_in[:], msc_t[:])
            nc.gpsimd.collective_compute(
                "AllReduce",
                mybir.AluOpType.add,
                replica_groups=[list(range(NCORES))],
                ins=[cc_in.opt()],
                outs=[cc_out.opt()],
            )
            msum_t = sp.tile([128, 32], F32, name=f"ms{r}", tag="msm")
            nc.sync.dma_start(msum_t[:], cc_out[:])
            m16_t = sp.tile([128, 32], F16, name=f"m16{r}", tag="m16")
            nc.vector.tensor_copy(out=m16_t[:], in_=msum_t[:])

        if mode == "cc":
            body = body_cc
        else:
            body = body_bfirst if order == "bfirst" else body_inter
        if loop_reps:
            with tc.For_i(0, loop_reps, 1):
                body(0)
        else:
            for r in range(reps):
                if serialize and r > 0:
                    tc.strict_bb_all_engine_barrier()
                body(r)

    nc.compile()
    return nc


def _pack_inputs(x, b, weights):
    xs = np.asarray(x, dtype=np.float32).reshape(B, CI, H, W)
    bs = np.asarray(b, dtype=np.float32).reshape(B, CI, H, W)
    ws = np.asarray(weights, dtype=np.float32).reshape(CI, CO, H, W)

    # one global fp8 scale, folded into the moving operands
    w_sc = float(np.abs(ws).max()) / F8MAX
    w8 = (ws / w_sc).astype(E3M4)                                 # [CI,CO,H,W]
    w8_t = np.transpose(w8, (0, 2, 3, 1))                         # [CI,H,W,CO]
    s_t = np.transpose((xs + bs) * w_sc, (1, 2, 3, 0)).astype(np.float16)
    b_t = np.transpose(bs * (w_sc * KB), (1, 2, 3, 0)).astype(E3M4)

    WC, SC = CHUNK_L * 128, CHUNK_L * 32
    in_maps = []
    for c in range(NCORES):
        h0, h1 = c * HL, (c + 1) * HL
        in_maps.append(
            {
                "w8": np.ascontiguousarray(w8_t[:, h0:h1].reshape(128, NCHUNK * WC)),
                "mvs": np.ascontiguousarray(s_t[:, h0:h1].reshape(128, NCHUNK * SC)),
                "mvb": np.ascontiguousarray(b_t[:, h0:h1].reshape(128, NCHUNK * SC)),
            }
        )
    return in_maps


def _unpack_output(res):
    out = np.empty((B, 1, CO, H, W), dtype=np.float32)
    for c in range(NCORES):
        o = res[c]["out"].astype(np.float32).reshape(128, HL, W, B)  # [j,hl,w,b]
        out[:, 0, :, c * HL : (c + 1) * HL, :] = np.transpose(o, (3, 0, 1, 2))
    return out


def kernel(x: np.ndarray, b: np.ndarray, weights: np.ndarray) -> np.ndarray:
    global LAST_EXEC_TIME_NS

    in_maps = _pack_inputs(x, b, weights)

    if "nc" not in _NC_CACHE:
        _NC_CACHE["nc"] = _build_nc(order=DEFAULT_ORDER)
    nc = _NC_CACHE["nc"]

    trace = os.environ.get("KERNEL_TRACE", "0") == "1"
    res = run_bass_kernel_spmd(nc, in_maps, list(range(NCORES)), trace=trace)
    LAST_EXEC_TIME_NS = res.exec_time_ns

    return _unpack_output(res.results)
